# revision 1
# baseline (speedup 1.0000x reference)
"""CASSI adjoint (gather shifted bands + mask) as a Bass/Tile SPMD kernel
on 8 Trainium2 NeuronCores.

Reference computation (shapes hardcoded for H=W=1024, L=28, PAD=32):
    out[0, l, h, w] = y_1hw[0, dy[l] + h, dx[l] + w] * mask2d[h, w]
with integer offsets dx/dy derived from phi_d_deg and s_nom on the host.

Sharding: the H (row) dimension is split across the 8 cores — every core
runs an identical program (all 28 bands, offsets baked in as compile-time
constants) over its own 128-row chunk of y/mask/out. Zero communication.
"""

import numpy as np

import concourse.bass as bass  # noqa: F401  (registers AP machinery)
import concourse.mybir as mybir
from concourse import bacc, tile
from concourse.bass_utils import run_bass_kernel_spmd

PI = 3.141592653589793

H, W, L = 1024, 1024, 28
HP, WP = 1056, 1056  # padded input extents (H+PAD, W+PAD)
NCORES = 8
RC = H // NCORES  # 128 rows per core

_cache: dict = {}


def _offsets(phi_d_deg, s_nom):
    """Integer dispersion offsets, mirroring the f32 arithmetic of the
    reference (round-half-to-even, then dynamic_slice start clamping)."""
    phi = np.float32(np.asarray(phi_d_deg, dtype=np.float32).reshape(-1)[0])
    phi_rad = np.float32(phi * np.float32(PI / 180.0))
    s = np.asarray(s_nom, dtype=np.float32)
    dx_f = (s * np.float32(np.cos(phi_rad))).astype(np.float32)
    dy_f = (s * np.float32(np.sin(phi_rad))).astype(np.float32)
    dx_f = (dx_f - dx_f.min()).astype(np.float32)
    dy_f = (dy_f - dy_f.min()).astype(np.float32)
    dx = np.round(dx_f).astype(np.int32)
    dy = np.round(dy_f).astype(np.int32)
    dx = np.clip(dx, 0, WP - W)
    dy = np.clip(dy, 0, HP - H)
    return dx, dy


def _build(dx, dy, group=4, obufs=3):
    """Build + compile the per-core program for the given band offsets."""
    max_dy = int(dy.max())
    nrows = RC + max_dy
    nc = bacc.Bacc("TRN2", target_bir_lowering=False, debug=False,
                   num_devices=NCORES)
    f32 = mybir.dt.float32
    y_in = nc.dram_tensor("y_loc", [nrows, WP], f32, kind="ExternalInput")
    m_in = nc.dram_tensor("mask_loc", [RC, W], f32, kind="ExternalInput")
    o_out = nc.dram_tensor("out_loc", [L, RC, W], f32, kind="ExternalOutput")

    with tile.TileContext(nc) as tc:
        with (
            tc.tile_pool(name="singles", bufs=1) as singles,
            tc.tile_pool(name="ob", bufs=obufs) as obp,
        ):
            ytiles = {}
            for d in sorted({int(v) for v in dy}):
                yt = singles.tile([RC, WP], f32, tag=f"y{d}", name=f"y{d}")
                nc.sync.dma_start(out=yt[:, :], in_=y_in[d : d + RC, :])
                ytiles[d] = yt
            mt = singles.tile([RC, W], f32, tag="mask", name="mask")
            nc.sync.dma_start(out=mt[:, :], in_=m_in[:, :])

            for g0 in range(0, L, group):
                gsz = min(group, L - g0)
                ot = obp.tile([RC, group * W], f32, tag="obuf", name=f"ob{g0}")
                for j in range(gsz):
                    l = g0 + j
                    ys = ytiles[int(dy[l])]
                    x0 = int(dx[l])
                    nc.vector.tensor_mul(
                        ot[:, j * W : (j + 1) * W], ys[:, x0 : x0 + W], mt[:, :]
                    )
                dview = o_out[g0 : g0 + gsz, :, :].rearrange("l h w -> h l w")
                sview = ot[:, : gsz * W].rearrange("h (l w) -> h l w", w=W)
                nc.sync.dma_start(out=dview, in_=sview)

    nc.compile()
    return nc


def _run(inputs, trace=False):
    y = np.ascontiguousarray(np.asarray(inputs["y_1hw"], dtype=np.float32)[0])
    mask = np.ascontiguousarray(np.asarray(inputs["mask2d"], dtype=np.float32))
    assert y.shape == (HP, WP) and mask.shape == (H, W)
    dx, dy = _offsets(inputs["phi_d_deg"], inputs["s_nom"])
    assert len(dx) == L

    key = (tuple(dx.tolist()), tuple(dy.tolist()))
    if key not in _cache:
        _cache[key] = _build(dx, dy)
    nc = _cache[key]

    max_dy = int(dy.max())
    in_maps = []
    for c in range(NCORES):
        h0 = c * RC
        in_maps.append(
            {
                "y_loc": np.ascontiguousarray(y[h0 : h0 + RC + max_dy, :]),
                "mask_loc": np.ascontiguousarray(mask[h0 : h0 + RC, :]),
            }
        )
    res = run_bass_kernel_spmd(nc, in_maps, core_ids=list(range(NCORES)),
                               trace=trace)
    out = np.empty((1, L, H, W), dtype=np.float32)
    for c in range(NCORES):
        out[0, :, c * RC : (c + 1) * RC, :] = res.results[c]["out_loc"]
    return out, res


def kernel(**inputs) -> np.ndarray:
    out, _ = _run(inputs)
    return out
